# revision 29
# baseline (speedup 1.0000x reference)
"""Multi-head attention (B=4, L=1024, D=1024, H=16) on 8 TRN2 NeuronCores.

Sharding: (batch, vd-half) — core c handles batch c//2 and value/head
dimension half c%2 (heads 8*(c%2) .. 8*(c%2)+7). Each core computes its
512-wide slice of the Q/K/V projections for ALL 1024 queries/keys (no
duplicated projection work), full attention for its 8 heads, and a partial
output projection out_part = (O/denom * q_mask) @ Wo[vd_half].  The host
sums the two partials per batch and adds bo and the folded-out V bias:
post-softmax attention rows sum to 1, so attn @ (xWv + 1*bv) =
attn @ xWv + bv, and the bv term becomes a host-side
q_mask (x) (bv @ Wo) rank-1 update.  No collectives.

Schedule: engine memsets first, PE warmup dummies, then ONLY head-pair
j0's Q/K projections (kt-outer, 2 sp tiles, streaming behind the
interleaved qT/Wq/xT/Wk DMAs) so the S/exp stream starts ~30us in.
j1-3 projections run as single-bank kt-inner passes paced into the
attention stages (LOOKAHEAD=3 keeps stage si+1's S from being traced
before its projections).  V projection and the qh0 output-projection
tiles are further PE fillers.  Tail: qh1 out tiles partially
pre-accumulated into freed S banks; tail epilogues on ScalarE.

  Q^T[vd, q] = Wq_h(lhsT) @ qT(rhs)  (+bq)
  K^T[vd, k] = Wk_h(lhsT) @ xT(rhs)  (+bk)
  V  [k, vd] = xT(lhsT) @ Wv_h(rhs)
  S^T[k, q]  = K^T_h(lhsT, K=64) @ Q^T_h for a head PAIR, row-packed into
               one [128, 2, 512] PSUM tile (concurrent via row groups)
  expS       = exp(S^T/8 + kmask_bias)   (ScalarE, PSUM->SBUF bf16)
  O^T+denom  = V_aug(lhsT, M=65) @ expS  (V cols + ones col per head)
  scale      = DVE cast denom rows -> PE K=1 broadcast of RAW denom ->
               wide reciprocal_approx_fast [128,512] -> fused multiply
  out_part[q, d] = (O^T_scaled.T @ Wo_h) * q_mask    (bf16)
"""

import os

os.environ.setdefault("MYCRO_LOCAL_CACHE", "1")

import numpy as np
import ml_dtypes

BF16 = ml_dtypes.bfloat16

B, LQ, LK = 4, 1024, 1024
D = 1024          # QD = KD = VD
H, DH = 16, 64
VH = 512          # vd half per core
NJ = 4            # vd-tiles (head pairs) per core
NCORES = 8
NEG = -1e4        # additive key-mask bias

_NC_CACHE = {}


def _build_nc():
    import concourse.bacc as bacc
    import concourse.mybir as mybir
    import concourse.tile as tile

    dt = mybir.dt

    nc = bacc.Bacc(
        "TRN2",
        debug=False,
        target_bir_lowering=False,
        num_devices=NCORES,
    )

    def din(name, shape, dtype):
        return nc.dram_tensor(name, shape, dtype, kind="ExternalInput").ap()

    aps = {
        "qT": din("qT", [D, LQ], dt.bfloat16),
        "xT": din("xT", [D, LK], dt.bfloat16),
        "Wq": din("Wq", [D, VH], dt.bfloat16),
        "Wk": din("Wk", [D, VH], dt.bfloat16),
        "Wv": din("Wv", [D, VH], dt.bfloat16),
        "Wo": din("Wo", [VH, D], dt.bfloat16),
        # packed per-partition constants: cols 0-3 bq, 4-7 bk, 8-15 kbias,
        # 16-23 q_mask (by query tile)
        "consts": din("consts", [128, 24], dt.float32),
        "out": nc.dram_tensor("out", [LQ, D], dt.bfloat16,
                              kind="ExternalOutput").ap(),
    }

    with tile.TileContext(nc) as tc:
        _body(tc, dt, mybir, aps)

    nc.compile()
    return nc


def _body(tc, dt, mybir, aps):
    from contextlib import ExitStack

    ALU = mybir.AluOpType
    AF = mybir.ActivationFunctionType
    nc = tc.nc
    with ExitStack() as ctx:
        const = ctx.enter_context(tc.tile_pool(name="const", bufs=1))
        espool = ctx.enter_context(tc.tile_pool(name="es", bufs=10))
        psum = ctx.enter_context(tc.tile_pool(name="psum", bufs=3, space="PSUM"))
        spair = ctx.enter_context(tc.tile_pool(name="spair", bufs=2, space="PSUM"))
        srp = ctx.enter_context(tc.tile_pool(name="srp", bufs=1, space="PSUM"))
        opool = ctx.enter_context(tc.tile_pool(name="osb", bufs=3))
        utp = ctx.enter_context(tc.tile_pool(name="utp", bufs=2))

        def ctile(shape, dtype, tag):
            return const.tile(shape, dtype, tag=tag, name=tag)

        # ---- memsets FIRST: the GpSimd queue must run these before any
        # of its DMA descriptors (engine queues are strict FIFO) ----
        ones1 = ctile([1, 128], dt.bfloat16, "ones1")
        nc.gpsimd.memset(ones1[:], 1.0)
        ones64 = ctile([1, 64], dt.bfloat16, "ones64")
        nc.gpsimd.memset(ones64[:], 1.0)
        scr = ctile([1, 512], dt.bfloat16, "scr")
        nc.gpsimd.memset(scr[:], 0.0)
        # V_aug tiles: per k-tile [128, 8*(64+1)]; local head h at cols
        # [65h, 65h+64), ones at 65h+64
        v_sb = [ctile([128, 8 * (DH + 1)], dt.bfloat16, f"v{t}")
                for t in range(8)]
        for t in range(8):
            ones_cols = v_sb[t][:].rearrange(
                "p (h c) -> p h c", c=DH + 1)[:, :, DH:DH + 1]
            nc.gpsimd.memset(ones_cols, 1.0)

        # ---- PE warmup dummies: no DMA deps, un-throttle HAM early ----
        dps = srp.tile([128, 512], dt.float32, tag="sr", name="sr")
        for _ in range(7):
            nc.tensor.matmul(dps[:], ones1[:], scr[:], start=True, stop=True)

        # ---- input DMAs: ordered by first use, 3 queues round-robin ----
        cst = ctile([128, 24], dt.float32, "cst")
        nc.sync.dma_start(cst[:], aps["consts"][:, :])

        engs = [nc.sync, nc.scalar, nc.gpsimd]
        ei = [0]
        tiles = {}

        def load(name, dram, nfree, kt):
            tl = ctile([128, nfree], dt.bfloat16, f"{name}_{kt}")
            view = dram.rearrange("(t p) n -> p t n", p=128)
            engs[ei[0] % 3].dma_start(tl[:, :], view[:, kt, :])
            ei[0] += 1
            tiles[(name, kt)] = tl

        for kt in range(8):
            load("qT", aps["qT"], LQ, kt)
            load("wq", aps["Wq"], VH, kt)
            load("xT", aps["xT"], LK, kt)
            load("wk", aps["Wk"], VH, kt)
        for kt in range(8):
            load("wv", aps["Wv"], VH, kt)
        for j in range(4):
            load("wo", aps["Wo"], D, j)

        t_of = lambda name, kt: tiles[(name, kt)]

        bq_c = lambda j: cst[:, j:j + 1]
        bk_c = lambda j: cst[:, 4 + j:5 + j]
        kb_c = lambda kt: cst[:, 8 + kt:9 + kt]
        qm_c = lambda qt: cst[:, 16 + qt:17 + qt]

        qTp = [ctile([128, LQ], dt.bfloat16, f"qTp{j}") for j in range(NJ)]
        kT_sb = [ctile([128, LK], dt.bfloat16, f"kT{j}") for j in range(NJ)]

        # ---- Q then K projections, kt-outer across all 8 banks (bank
        # rotation hides accumulation drains; PE stays dense behind the
        # DMA stream).  The 8 accumulators map onto the attention pools'
        # rings so ring reuse gives per-tile WAR deps, no barrier.
        def projection(wname, src, dst_list, bias_of):
            acc = {}
            for j in range(2):
                spj = spair.tile([128, 2, 512], dt.float32, tag="sp", name="sp")
                acc[(j, 0)], acc[(j, 1)] = spj[:, 0, :], spj[:, 1, :]
            for key in ((2, 0), (2, 1), (3, 0)):
                acc[key] = psum.tile([128, 512], dt.float32,
                                     tag="ps", name="ps")[:]
            acc[(3, 1)] = srp.tile([128, 512], dt.float32,
                                   tag="sr", name="sr")[:]
            for kt in range(8):
                for j in range(NJ):
                    for n in range(2):
                        nc.tensor.matmul(
                            acc[(j, n)],
                            t_of(wname, kt)[:, 128 * j:128 * (j + 1)],
                            t_of(src, kt)[:, 512 * n:512 * (n + 1)],
                            start=(kt == 0), stop=(kt == 7))
            for j in range(NJ):
                for n in range(2):
                    nc.vector.tensor_scalar_add(
                        dst_list[j][:, 512 * n:512 * (n + 1)],
                        acc[(j, n)], bias_of(j))

        projection("wq", "qT", qTp, bq_c)
        projection("wk", "xT", kT_sb, bk_c)

        def v_proj(t):
            ps = psum.tile([128, 512], dt.float32, tag="ps", name="ps")
            for kd in range(8):
                nc.tensor.matmul(
                    ps[:], t_of("xT", kd)[:, 128 * t:128 * (t + 1)],
                    t_of("wv", kd)[:], start=(kd == 0), stop=(kd == 7))
            vout = v_sb[t][:].rearrange(
                "p (h c) -> p h c", c=DH + 1)[:, :, 0:DH]
            vin = ps[:].rearrange("p (i c) -> p i c", c=DH)
            nc.vector.tensor_copy(vout, vin)

        # stage order: qh-outer, j-inner.  stage si -> (qh, j) with j == si%4
        STAGES = [(qh, j) for qh in range(2) for j in range(NJ)]
        oTs = [ctile([128, LQ], dt.bfloat16, f"oTs{j}") for j in range(NJ)]
        dnb = ctile([1, 2, 512], dt.bfloat16, "dnb")
        rsr = ctile([128, 512], dt.float32, "rsr")

        es_tiles = {}

        def s_stage(si, kt):
            qh, j = STAGES[si]
            qc = slice(512 * qh, 512 * (qh + 1))
            kc = slice(128 * kt, 128 * (kt + 1))
            sp = spair.tile([128, 2, 512], dt.float32, tag="sp", name="sp")
            nc.tensor.matmul(sp[:, 0, :], kT_sb[j][0:64, kc],
                             qTp[j][0:64, qc], start=True, stop=True)
            nc.tensor.matmul(sp[:, 1, :], kT_sb[j][64:128, kc],
                             qTp[j][64:128, qc], start=True, stop=True)
            es = espool.tile([128, 2, 512], dt.bfloat16, tag="es", name="es")
            nc.scalar.activation(es[:], sp[:], AF.Exp,
                                 bias=kb_c(kt), scale=0.125)
            es_tiles[(si, kt)] = es

        def o_stage(si, kt, oA, oB):
            qh, j = STAGES[si]
            hA, hB = 2 * j, 2 * j + 1
            es = es_tiles.pop((si, kt))
            nc.tensor.matmul(oA[0:65, :], v_sb[kt][:, 65 * hA:65 * hA + 65],
                             es[:, 0, :], start=(kt == 0), stop=(kt == 7))
            nc.tensor.matmul(oB[0:65, :], v_sb[kt][:, 65 * hB:65 * hB + 65],
                             es[:, 1, :], start=(kt == 0), stop=(kt == 7))

        # S/exp stream lookahead: the sp ring is only 2 deep, so tracing S
        # further ahead than ~3 head-of-line-blocks the in-order PE queue
        # on exp drains (costs whole-stage stalls at stage boundaries)
        LOOKAHEAD = 3
        flat = [(si, kt) for si in range(8) for kt in range(8)]
        s_cursor = [0]

        def advance_s(upto):
            while s_cursor[0] < min(upto, 64):
                s_stage(*flat[s_cursor[0]])
                s_cursor[0] += 1

        # out-projection tiles (qt, n); qt 0-3 need qh0 chains, 4-7 qh1
        def out_tile(qt, n):
            qr = slice(128 * qt, 128 * (qt + 1))
            c = slice(512 * n, 512 * (n + 1))
            ps = psum.tile([128, 512], dt.float32, tag="ps", name="ps")
            for j in range(NJ):
                nc.tensor.matmul(ps[:], oTs[j][:, qr], t_of("wo", j)[:, c],
                                 start=(j == 0), stop=(j == NJ - 1))
            ot = opool.tile([128, 512], dt.bfloat16, tag="osb", name="osb")
            nc.vector.tensor_scalar_mul(ot[:], ps[:], qm_c(qt))
            nc.sync.dma_start(aps["out"][qr, c], ot[:])

        fillers = []
        fcursor = [0]

        def run_filler(k=1):
            for _ in range(k):
                if fcursor[0] < len(fillers):
                    fillers[fcursor[0]]()
                    fcursor[0] += 1

        def o_alloc():
            oA = psum.tile([128, 512], dt.float32, tag="ps", name="ps")
            oB = psum.tile([128, 512], dt.float32, tag="ps", name="ps")
            return oA, oB

        # prologue: S runway interleaved with the first four V tiles
        advance_s(1)
        v_proj(0)
        advance_s(2)
        v_proj(1)
        advance_s(3)
        v_proj(2)
        advance_s(4)
        v_proj(3)

        cur = o_alloc()
        for si in range(8):
            qh, j = STAGES[si]
            if si == 4:
                for qt in range(4):
                    for n in range(2):
                        fillers.append(lambda qt=qt, n=n: out_tile(qt, n))
            oA, oB = cur
            for kt in range(8):
                advance_s(8 * si + kt + 1 + LOOKAHEAD)
                o_stage(si, kt, oA, oB)
                if si == 0 and kt < 4:
                    v_proj(kt + 4)
                elif kt in (1, 3, 5):
                    run_filler()
            if si < 7:
                cur = o_alloc()

            if si == 7:
                # pre-accumulate j0-2 of four qh1 out tiles into the freed
                # S PSUM banks while the last scale chain runs
                parts = {}
                for n in range(2):
                    pt = spair.tile([128, 2, 512], dt.float32,
                                    tag="sp", name="sp")
                    for half, qt in enumerate((4, 5)):
                        ps = pt[:, half, :]
                        qr = slice(128 * qt, 128 * (qt + 1))
                        c = slice(512 * n, 512 * (n + 1))
                        for jj in range(3):
                            nc.tensor.matmul(
                                ps, oTs[jj][:, qr], t_of("wo", jj)[:, c],
                                start=(jj == 0), stop=False)
                        parts[(qt, n)] = ps

            # scale chain: cast raw denom rows -> broadcast -> wide recip
            oTu = utp.tile([128, 512], dt.bfloat16, tag="oTu", name="oTu")
            nc.vector.tensor_copy(dnb[:, 0, :], oA[64:65, :])
            nc.vector.tensor_copy(oTu[0:64, :], oA[0:64, :])
            nc.vector.tensor_copy(dnb[:, 1, :], oB[64:65, :])
            nc.vector.tensor_copy(oTu[64:128, :], oB[0:64, :])
            sr = srp.tile([128, 512], dt.float32, tag="sr", name="sr")
            nc.tensor.matmul(sr[0:64, :], ones64[:], dnb[:, 0, :],
                             start=True, stop=True)
            nc.tensor.matmul(sr[64:128, :], ones64[:], dnb[:, 1, :],
                             start=True, stop=True, tile_position=(0, 64))
            run_filler()
            nc.vector.reciprocal_approx_fast(out=rsr[:], in_=sr[:])
            if si < 7:
                nc.vector.scalar_tensor_tensor(
                    oTs[j][:, 512 * qh:512 * (qh + 1)], oTu[:], 0.0, rsr[:],
                    op0=ALU.bypass, op1=ALU.mult)

        # stage 7 epilogue: chunked stt so tail out tiles start ASAP;
        # tail PSUM->SBUF moves on the now-idle ScalarE
        for qt in range(4, 8):
            qr = slice(128 * qt, 128 * (qt + 1))
            lr = slice(128 * (qt - 4), 128 * (qt - 3))
            nc.vector.scalar_tensor_tensor(
                oTs[3][:, qr], oTu[:, lr], 0.0, rsr[:, lr],
                op0=ALU.bypass, op1=ALU.mult)
            for n in range(2):
                c = slice(512 * n, 512 * (n + 1))
                if (qt, n) in parts:
                    ps = parts.pop((qt, n))
                    nc.tensor.matmul(ps, oTs[3][:, qr], t_of("wo", 3)[:, c],
                                     start=False, stop=True)
                else:
                    pt = psum.tile([128, 512], dt.float32, tag="ps", name="ps")
                    for jj in range(NJ):
                        nc.tensor.matmul(
                            pt[:], oTs[jj][:, qr], t_of("wo", jj)[:, c],
                            start=(jj == 0), stop=(jj == NJ - 1))
                    ps = pt[:]
                ot = opool.tile([128, 512], dt.bfloat16, tag="osb", name="osb")
                nc.scalar.activation(ot[:], ps, AF.Copy, scale=qm_c(qt))
                nc.sync.dma_start(aps["out"][qr, c], ot[:])
        run_filler(len(fillers))


def get_nc():
    if "nc" not in _NC_CACHE:
        _NC_CACHE["nc"] = _build_nc()
    return _NC_CACHE["nc"]


def make_in_maps(q, x, q_mask, k_mask, Wq, bq, Wk, bk, Wv, bv, Wo, bo):
    """Host-side shard/layout prep. Returns in_maps for cores 0..7."""
    in_maps = []
    for c in range(NCORES):
        b, hf = c // 2, c % 2
        vs = slice(VH * hf, VH * (hf + 1))
        kbias = np.where(k_mask[b] != 0, 0.0, NEG).astype(np.float32)
        consts = np.empty((128, 24), np.float32)
        consts[:, 0:4] = np.asarray(bq, np.float32)[vs].reshape(4, 128).T
        consts[:, 4:8] = np.asarray(bk, np.float32)[vs].reshape(4, 128).T
        consts[:, 8:16] = kbias.reshape(8, 128).T
        consts[:, 16:24] = q_mask[b].astype(np.float32).reshape(8, 128).T
        in_maps.append({
            "qT": np.ascontiguousarray(q[b].T).astype(BF16),
            "xT": np.ascontiguousarray(x[b].T).astype(BF16),
            "Wq": np.ascontiguousarray(np.asarray(Wq)[:, vs]).astype(BF16),
            "Wk": np.ascontiguousarray(np.asarray(Wk)[:, vs]).astype(BF16),
            "Wv": np.ascontiguousarray(np.asarray(Wv)[:, vs]).astype(BF16),
            "Wo": np.ascontiguousarray(np.asarray(Wo)[vs, :]).astype(BF16),
            "consts": np.ascontiguousarray(consts),
        })
    return in_maps


def combine_outputs(results, q_mask, bv, Wo, bo):
    """Sum the two vd-half partials per batch; add bo and the folded
    V-bias term q_mask (x) (bv @ Wo) (post-softmax rows sum to 1)."""
    bvWo = np.asarray(bv, np.float32) @ np.asarray(Wo, np.float32)
    bo = np.asarray(bo, np.float32)
    out = np.empty((B, LQ, D), np.float32)
    for b in range(B):
        base = (results[2 * b]["out"].astype(np.float32)
                + results[2 * b + 1]["out"].astype(np.float32))
        out[b] = (base + q_mask[b].astype(np.float32)[:, None] * bvWo[None, :]
                  + bo[None, :])
    return out


def kernel(q, x, q_mask, k_mask, Wq, bq, Wk, bk, Wv, bv, Wo, bo):
    from concourse import bass_utils

    q = np.asarray(q, np.float32)
    x = np.asarray(x, np.float32)
    q_mask = np.asarray(q_mask)
    k_mask = np.asarray(k_mask)

    nc = get_nc()
    in_maps = make_in_maps(q, x, q_mask, k_mask, Wq, bq, Wk, bk, Wv, bv, Wo, bo)
    res = bass_utils.run_bass_kernel_spmd(nc, in_maps, core_ids=list(range(NCORES)))
    return combine_outputs(res.results, q_mask, bv, Wo, bo)
